# revision 77
# baseline (speedup 1.0000x reference)
"""Trainium2 Bass kernel for the attention layer:

    proj = encoder @ Wa                       [B, Te, H]
    A    = einsum('beh,bdh->bed', proj, dec)  [B, Te, Td]
    A    = exp(A) * mask[:, :, None]
    A    = A.transpose(0, 2, 1);  A /= A.sum(axis=2, keepdims=True)
    C    = einsum('bde,beh->bdh', A, encoder) [B, Td, H]
    out  = concat([C, decoder], axis=2)       [B, Td, 2H]

Sharding: data-parallel over batch B across 8 NeuronCores (8 batches/core).

Per-core dataflow (per batch b, everything fp32 in HBM/SBUF):
  - Load enc[b] into 4 SBUF tiles [128, 257] (col 256 memset to 1.0 -> the
    softmax denominator falls out of the last matmul for free).
  - Load dec[b] into the right half of 4 output tiles [128, 512] (cols
    256:512) -- this doubles as the concat copy.
  - PE-transpose enc/dec into encT/decT [h, seq] (16 [128,128] transposes).
  - projT[g,e] = Wa^T @ encT   (lhsT=Wa chunks,    rhs=encT)  2 psum tiles
  - A[e,d]     = projT^T @ decT (lhsT=projT slices, rhs=decT)  4 psum tiles
  - expS = exp(A + ln(mask) - 64*ln2)  on ScalarE (mask transposed once via
    PE).  The 2^-64 shift keeps the deferred-normalization matmul finite:
    without it, max exp(A) = 2.3e38 and products exp(A)*enc overflow fp32
    even though the reference (which normalizes first) stays finite.
  - C_un[d, 0:257] = expS^T @ [enc | 1]  (lhsT=expS slices, rhs=enc_aug)
    -> col 256 = S_s[d] = sum_e expS[e,d]  (the 2^-64 cancels in the ratio)
  - NaN fidelity: reference exp(A) hits inf at 2 positions (margin >1.7 in
    A), making those 2 (b,d) rows NaN.  S_u = S_s * 2^64 overflows to inf
    exactly for those rows (largest finite sum is 0.68*f32max, smallest
    overflowed is 5.9x f32max); zS = 0 * S_u is NaN there and +0 elsewhere.
  - C[d] = C_un[d,0:256] * recip(S_s[d]) + zS[d] in one DVE tensor_scalar,
    written into the output tile's left half; DMA full [128,512] rows out.

Matmuls run with operands bitcast to float32r (FP22 multiply / FP32
accumulate, 4x the fp32 PE throughput). Set MM_DT = F32 for full fp32.
"""

import math

import numpy as np

import bass_rust
import concourse.bass as bass
import concourse.mybir as mybir
import concourse.tile as tile
from concourse.bass_utils import run_bass_kernel_spmd
from concourse.masks import make_identity
from concourse.vector_clock import ScopedClock

B, TE, TD, H = 64, 512, 512, 256
N_CORES = 8
B_LOC = B // N_CORES  # batches per core

F32 = mybir.dt.float32
F32R = mybir.dt.float32r
MM_DT = F32R  # matmul operand dtype (F32R: FP22 mul / FP32 acc, 4x faster)

P = 128
NE = TE // P  # 4 encoder-position tiles
ND = TD // P  # 4 decoder-position tiles
NH = H // P   # 2 feature chunks
ACT = mybir.ActivationFunctionType

SHIFT_POW = 64  # exp computed as exp(x - SHIFT_POW*ln2); undone via *2^64
SHIFT = SHIFT_POW * math.log(2.0)


def _patched_drain_and_barrier(self, tick_clock, wait_clock):
    # Stock Tile piles every end-of-kernel wait onto a single Drain; this
    # walrus build caps non-EventSemaphore instructions at one sync wait.
    # Split the waits across a chain of drains on SP (same semantics: all
    # waits retire before the final barrier + semaphore reset).
    nc = self.nc
    drain_inst = nc.sync.drain()
    wait_clock.add_sem_waits(
        drain_inst.ins, ScopedClock({None: tick_clock.global_clock})
    )
    si = drain_inst.ins.sync_info
    if si is not None and si.on_wait is not None and len(si.on_wait) > 1:
        waits = list(si.on_wait)
        drain_inst.ins.sync_info = bass_rust.SyncInfo(
            on_wait=[waits[0]], on_update=list(si.on_update or [])
        )
        # Round-robin the remaining waits across all engine queues so they
        # retire in parallel; the all_engine_barrier below joins them.
        engs = [nc.sync, nc.vector, nc.scalar, nc.tensor, nc.gpsimd]
        for i, w in enumerate(waits[1:]):
            d2 = engs[i % len(engs)].drain()
            d2.ins.sync_info = bass_rust.SyncInfo(on_wait=[w], on_update=[])

    nc.all_engine_barrier()
    assert self.sems is not None
    popped = nc._tile_sem_poison_stack.pop()
    assert popped is self._sem_poison
    nc.clear_and_free_semaphores(list(self.sems.allocated().values()))
    nc.all_engine_barrier()


tile.TileContext._drain_and_barrier = _patched_drain_and_barrier

_WAIT_CAP = {"EventSemaphore": 2}  # walrus wait-slot capacity by opcode
_wsplit_n = 0


def _split_multi_waits(nc: bass.Bass):
    """This walrus build allows 1 sync wait per instruction (2 for
    EventSemaphore), but Tile emits instructions with up to 3.  Move excess
    waits onto NoOps inserted just before the instruction on the same engine
    queue — identical ordering semantics, codegen-legal."""
    global _wsplit_n
    for bb in nc.m.functions[0].blocks:
        out = []
        for inst in bb.instructions:
            si = inst.sync_info
            waits = list(si.on_wait) if si is not None and si.on_wait else []
            cap = _WAIT_CAP.get(inst.opcode, 1)
            if len(waits) > cap:
                for w in waits[:-cap]:
                    _wsplit_n += 1
                    out.append(
                        mybir.InstNoOp(
                            name=f"I-wsplit-{_wsplit_n}",
                            sync_info=mybir.SyncInfo(on_wait=[w], on_update=[]),
                            bass_nofuse=True,
                            engine=inst.engine,
                        )
                    )
                inst.sync_info = mybir.SyncInfo(
                    on_wait=waits[-cap:], on_update=list(si.on_update or [])
                )
            out.append(inst)
        bb.instructions = out


def build_nc() -> bass.Bass:
    nc = bass.Bass()
    enc_d = nc.declare_dram_parameter("encoder", [B_LOC, TE, H], F32, isOutput=False)
    dec_d = nc.declare_dram_parameter("decoder", [B_LOC, TD, H], F32, isOutput=False)
    mask_d = nc.declare_dram_parameter("mask", [B_LOC, TE], F32, isOutput=False)
    wa_d = nc.declare_dram_parameter("Wa", [H, H], F32, isOutput=False)
    out_d = nc.declare_dram_parameter("out", [B_LOC, TD, 2 * H], F32, isOutput=True)

    with tile.TileContext(nc) as tc:
        with (
            tc.tile_pool(name="consts", bufs=1) as consts,
            tc.tile_pool(name="enc_sb", bufs=6) as enc_pool,
            tc.tile_pool(name="out_sb", bufs=4) as out_pool,
            tc.tile_pool(name="tr_sb", bufs=8 * NH) as tr_pool,
            tc.tile_pool(name="projT_sb", bufs=4 * NH) as projT_pool,
            tc.tile_pool(name="expa_sb", bufs=8 * NE) as expa_pool,
            tc.tile_pool(name="small_sb", bufs=4 * ND) as small_pool,
            tc.tile_pool(name="ps_tr", bufs=3, space="PSUM") as ps_tr,
            tc.tile_pool(name="ps_proj", bufs=1, space="PSUM") as ps_proj,
            tc.tile_pool(name="ps_a", bufs=2, space="PSUM") as ps_a,
            tc.tile_pool(name="ps_c", bufs=2, space="PSUM") as ps_c,
        ):
            EW = H + 2  # 258: encoder chunk width incl. ones cols
            loaded = {}

            def stage_load(b, prio=False):
                # encoder mega-tile [128, 4*258]: chunk t at cols t*258,
                # data cols 0:256, ones at 256:258. One DMA + one strided
                # ones fill per batch.
                enc_big = enc_pool.tile([P, NE * EW], F32R, tag="enc")
                enc_v = enc_big.rearrange("p (t c) -> p t c", c=EW)
                nc.scalar.dma_start(
                    out=enc_v[:, :, 0:H],
                    in_=enc_d[b].rearrange("(t p) h -> p t h", p=P).bitcast(F32R),
                )
                # output mega-tile [128, 4*512]: d-tile at cols d*512; right
                # half of each = decoder (the concat).
                out_big = out_pool.tile([P, ND * 2 * H], F32R, tag="out")
                out_v = out_big.rearrange("p (d c) -> p d c", c=2 * H)
                nc.sync.dma_start(
                    out=out_v[:, :, H : 2 * H],
                    in_=dec_d[b].rearrange("(d p) h -> p d h", p=P).bitcast(F32R),
                )
                loaded[b] = (enc_big, out_big)

            stage_load(0, prio=True)
            stage_load(1)

            # ---- constants ----
            ident = consts.tile([P, P], F32, tag="ident")
            make_identity(nc, ident)
            # f32r variants (GPSIMD memset/affine_select can't emit f32r;
            # DVE copy rounds f32 -> f32r legally)
            ident_r = consts.tile([P, P], F32R, tag="ident_r")
            nc.vector.tensor_copy(out=ident_r, in_=ident)
            ones_f = consts.tile([P, 2 * NE], F32, tag="ones_f")
            nc.gpsimd.memset(ones_f, 1.0)
            ones8_r = consts.tile([P, 2 * NE], F32R, tag="ones8_r")
            nc.vector.tensor_copy(out=ones8_r, in_=ones_f)

            wa_sb = []
            for k in range(NH):
                w = consts.tile([P, H], F32R, tag=f"wa{k}")
                nc.sync.dma_start(
                    out=w, in_=wa_d[k * P : (k + 1) * P, :].bitcast(F32R)
                )
                wa_sb.append(w)

            # mask -> transposed -> ln(mask); col t*B_LOC + b holds
            # ln(mask[b, t*128 : (t+1)*128]) as a [128,1] column.
            mask_raw = consts.tile([B_LOC, TE], F32, tag="mask_raw")
            nc.scalar.dma_start(out=mask_raw, in_=mask_d[:, :])
            mask_raw = consts.tile([B_LOC, TE], F32, tag="mask_raw")
            nc.sync.dma_start(out=mask_raw, in_=mask_d[:, :])
            mask_ps = ps_tr.tile([P, NE * B_LOC], F32, tag="tr")
            for t in range(NE):
                nc.tensor.transpose(
                    out=mask_ps[:, t * B_LOC : (t + 1) * B_LOC],
                    in_=mask_raw[:, t * P : (t + 1) * P],
                    identity=ident[:B_LOC, :B_LOC],
                )
            mask_ln = consts.tile([P, NE * B_LOC], F32, tag="mask_ln")
            nc.scalar.activation(out=mask_ln, in_=mask_ps, func=ACT.Ln)
            # fold in the -64*ln2 exp shift
            nc.vector.tensor_scalar_add(out=mask_ln, in0=mask_ln, scalar1=-SHIFT)

            # ---- per-batch pipeline ----
            # Emission is software-pipelined: batch b's front end (loads,
            # transposes, proj/score matmuls, exp) is emitted before batch
            # b-1's tail (context matmul, normalize, stores), so no engine
            # queue has a late-dependency op ahead of the next batch's
            # PE-feeding work (engine queues are strict FIFO).
            state = {}

            def stage_front(b):
                enc_big, out_big = loaded.pop(b)
                enc_v = enc_big.rearrange("p (t c) -> p t c", c=EW)
                nc.vector.tensor_copy(out=enc_v[:, :, H:EW], in_=ones8_r)
                enc_t = [enc_big[:, t * EW : (t + 1) * EW] for t in range(NE)]

                # transposes: encT/decT [h, seq] as NH tiles [128, 512].
                # All in f32r views (1.5 cyc/row vs 2.0 for f32; the values
                # feed f32r matmuls and would be FP22-truncated there anyway).
                encT, decT = [], []
                for which in ("enc", "dec"):
                    for k in range(NH):
                        ps = ps_tr.tile([P, TE], F32, tag="tr")
                        for t in range(NE):
                            if which == "enc":
                                src_ap = enc_big[:, t * EW + k * P : t * EW + (k + 1) * P]
                                out_ap = ps[:, t * P : (t + 1) * P].bitcast(F32R)
                                idn = ident_r
                            else:
                                src_ap = out_big[
                                    :, t * 2 * H + H + k * P : t * 2 * H + H + (k + 1) * P
                                ]
                                out_ap = ps[:, t * P : (t + 1) * P].bitcast(F32R)
                                idn = ident_r
                            nc.tensor.transpose(out=out_ap, in_=src_ap, identity=idn)
                        sb = tr_pool.tile([P, TE], F32R, tag="tr")
                        if which == "enc":
                            nc.vector.tensor_copy(out=sb, in_=ps.bitcast(F32R))
                            encT.append(sb)
                        else:
                            nc.scalar.copy(out=sb, in_=ps.bitcast(F32R))
                            decT.append(sb)

                # projT[g, e] = Wa^T @ encT  (accumulate over h chunks)
                projT = []
                for g in range(NH):
                    pps = ps_proj.tile([P, TE], F32, tag="proj")
                    for k in range(NH):
                        nc.tensor.matmul(
                            out=pps,
                            lhsT=wa_sb[k][:, g * P : (g + 1) * P],
                            rhs=encT[k],
                            start=(k == 0),
                            stop=(k == NH - 1),
                        )
                    sb = projT_pool.tile([P, TE], F32R, tag="proj")
                    if g == 0:
                        nc.vector.tensor_copy(out=sb, in_=pps)
                    else:
                        nc.scalar.copy(out=sb, in_=pps)
                    projT.append(sb)

                # A[e, d] = projT^T @ decT;  expS = exp(A + ln(mask) - shift)
                expa = []
                for t in range(NE):
                    aps = ps_a.tile([P, TD], F32, tag="a")
                    for k in range(NH):
                        nc.tensor.matmul(
                            out=aps,
                            lhsT=projT[k][:, t * P : (t + 1) * P],
                            rhs=decT[k],
                            start=(k == 0),
                            stop=(k == NH - 1),
                        )
                    ea = expa_pool.tile([P, TD], F32R, tag="expa")
                    col = t * B_LOC + b
                    nc.scalar.activation(
                        out=ea,
                        in_=aps,
                        func=ACT.Exp,
                        bias=mask_ln[:, col : col + 1],
                    )
                    expa.append(ea)
                state[b] = (enc_t, out_big, expa)

            def stage_tail(b):
                enc_t, out_big, expa = state.pop(b)
                last = b == B_LOC - 1
                if last:
                    # the dec halves are ready as soon as the load landed;
                    # storing them now halves the end-of-kernel store burst
                    for d in range(ND):
                        nc.sync.dma_start(
                            out=out_d[b, d * P : (d + 1) * P, H : 2 * H].bitcast(F32R),
                            in_=out_big[:, d * 2 * H + H : (d + 1) * 2 * H],
                        )
                # C_un[d, 0:258] = expS^T @ [enc | 1]; normalize; emit output
                for d in range(ND):
                    cps = ps_c.tile([P, H + 2], F32, tag="c")
                    for t in range(NE):
                        nc.tensor.matmul(
                            out=cps,
                            lhsT=expa[t][:, d * P : (d + 1) * P],
                            rhs=enc_t[t],
                            start=(t == 0),
                            stop=(t == NE - 1),
                        )
                    rec = small_pool.tile([P, 1], F32, tag="rec")
                    nc.vector.reciprocal(out=rec, in_=cps[:, H : H + 1])
                    # zS = (S_s * 2^64) * 0: the first mult overflows to inf
                    # exactly where the reference's unshifted sum does, the
                    # second turns inf -> NaN and finite -> +0.
                    zs = small_pool.tile([P, 1], F32, tag="zs")
                    nc.vector.tensor_scalar(
                        out=zs,
                        in0=cps[:, H : H + 1],
                        scalar1=2.0**SHIFT_POW,
                        scalar2=0.0,
                        op0=mybir.AluOpType.mult,
                        op1=mybir.AluOpType.mult,
                    )
                    nc.vector.tensor_scalar(
                        out=out_big[:, d * 2 * H : d * 2 * H + H],
                        in0=cps[:, 0:H],
                        scalar1=rec[:, 0:1],
                        scalar2=zs[:, 0:1],
                        op0=mybir.AluOpType.mult,
                        op1=mybir.AluOpType.add,
                    )
                    if last:
                        nc.sync.dma_start(
                            out=out_d[b, d * P : (d + 1) * P, 0:H].bitcast(F32R),
                            in_=out_big[:, d * 2 * H : d * 2 * H + H],
                        )
                    else:
                        nc.sync.dma_start(
                            out=out_d[b, d * P : (d + 1) * P, :].bitcast(F32R),
                            in_=out_big[:, d * 2 * H : (d + 1) * 2 * H],
                        )

            stage_front(0)
            for b in range(1, B_LOC):
                if b + 1 < B_LOC:
                    stage_load(b + 1)
                stage_front(b)
                stage_tail(b - 1)
            stage_tail(B_LOC - 1)

    _split_multi_waits(nc)
    return nc


_NC_CACHE = None


def _get_nc():
    global _NC_CACHE
    if _NC_CACHE is None:
        _NC_CACHE = build_nc()
    return _NC_CACHE


def kernel(encoder, decoder, mask, Wa):
    encoder = np.ascontiguousarray(encoder, dtype=np.float32)
    decoder = np.ascontiguousarray(decoder, dtype=np.float32)
    mask = np.ascontiguousarray(mask, dtype=np.float32)
    Wa = np.ascontiguousarray(Wa, dtype=np.float32)

    nc = _get_nc()
    in_maps = []
    for c in range(N_CORES):
        s = slice(c * B_LOC, (c + 1) * B_LOC)
        in_maps.append(
            {
                "encoder": encoder[s],
                "decoder": decoder[s],
                "mask": mask[s],
                "Wa": Wa,
            }
        )
    res = run_bass_kernel_spmd(nc, in_maps, list(range(N_CORES)))
    return np.concatenate([res.results[c]["out"] for c in range(N_CORES)], axis=0)


# revision 80
# speedup vs baseline: 1.0044x; 1.0044x over previous
"""Trainium2 Bass kernel for the attention layer:

    proj = encoder @ Wa                       [B, Te, H]
    A    = einsum('beh,bdh->bed', proj, dec)  [B, Te, Td]
    A    = exp(A) * mask[:, :, None]
    A    = A.transpose(0, 2, 1);  A /= A.sum(axis=2, keepdims=True)
    C    = einsum('bde,beh->bdh', A, encoder) [B, Td, H]
    out  = concat([C, decoder], axis=2)       [B, Td, 2H]

Sharding: data-parallel over batch B across 8 NeuronCores (8 batches/core).

Per-core dataflow (per batch b, everything fp32 in HBM/SBUF):
  - Load enc[b] into 4 SBUF tiles [128, 257] (col 256 memset to 1.0 -> the
    softmax denominator falls out of the last matmul for free).
  - Load dec[b] into the right half of 4 output tiles [128, 512] (cols
    256:512) -- this doubles as the concat copy.
  - PE-transpose enc/dec into encT/decT [h, seq] (16 [128,128] transposes).
  - projT[g,e] = Wa^T @ encT   (lhsT=Wa chunks,    rhs=encT)  2 psum tiles
  - A[e,d]     = projT^T @ decT (lhsT=projT slices, rhs=decT)  4 psum tiles
  - expS = exp(A + ln(mask) - 64*ln2)  on ScalarE (mask transposed once via
    PE).  The 2^-64 shift keeps the deferred-normalization matmul finite:
    without it, max exp(A) = 2.3e38 and products exp(A)*enc overflow fp32
    even though the reference (which normalizes first) stays finite.
  - C_un[d, 0:257] = expS^T @ [enc | 1]  (lhsT=expS slices, rhs=enc_aug)
    -> col 256 = S_s[d] = sum_e expS[e,d]  (the 2^-64 cancels in the ratio)
  - NaN fidelity: reference exp(A) hits inf at 2 positions (margin >1.7 in
    A), making those 2 (b,d) rows NaN.  S_u = S_s * 2^64 overflows to inf
    exactly for those rows (largest finite sum is 0.68*f32max, smallest
    overflowed is 5.9x f32max); zS = 0 * S_u is NaN there and +0 elsewhere.
  - C[d] = C_un[d,0:256] * recip(S_s[d]) + zS[d] in one DVE tensor_scalar,
    written into the output tile's left half; DMA full [128,512] rows out.

Matmuls run with operands bitcast to float32r (FP22 multiply / FP32
accumulate, 4x the fp32 PE throughput). Set MM_DT = F32 for full fp32.
"""

import math

import numpy as np

import bass_rust
import concourse.bass as bass
import concourse.mybir as mybir
import concourse.tile as tile
from concourse.bass_utils import run_bass_kernel_spmd
from concourse.masks import make_identity
from concourse.vector_clock import ScopedClock

B, TE, TD, H = 64, 512, 512, 256
N_CORES = 8
B_LOC = B // N_CORES  # batches per core

F32 = mybir.dt.float32
F32R = mybir.dt.float32r
MM_DT = F32R  # matmul operand dtype (F32R: FP22 mul / FP32 acc, 4x faster)

P = 128
NE = TE // P  # 4 encoder-position tiles
ND = TD // P  # 4 decoder-position tiles
NH = H // P   # 2 feature chunks
ACT = mybir.ActivationFunctionType

SHIFT_POW = 64  # exp computed as exp(x - SHIFT_POW*ln2); undone via *2^64
SHIFT = SHIFT_POW * math.log(2.0)


def _patched_drain_and_barrier(self, tick_clock, wait_clock):
    # Stock Tile piles every end-of-kernel wait onto a single Drain; this
    # walrus build caps non-EventSemaphore instructions at one sync wait.
    # Split the waits across a chain of drains on SP (same semantics: all
    # waits retire before the final barrier + semaphore reset).
    nc = self.nc
    drain_inst = nc.sync.drain()
    wait_clock.add_sem_waits(
        drain_inst.ins, ScopedClock({None: tick_clock.global_clock})
    )
    si = drain_inst.ins.sync_info
    if si is not None and si.on_wait is not None and len(si.on_wait) > 1:
        waits = list(si.on_wait)
        drain_inst.ins.sync_info = bass_rust.SyncInfo(
            on_wait=[waits[0]], on_update=list(si.on_update or [])
        )
        # Round-robin the remaining waits across all engine queues so they
        # retire in parallel; the all_engine_barrier below joins them.
        engs = [nc.sync, nc.vector, nc.scalar, nc.tensor, nc.gpsimd]
        for i, w in enumerate(waits[1:]):
            d2 = engs[i % len(engs)].drain()
            d2.ins.sync_info = bass_rust.SyncInfo(on_wait=[w], on_update=[])

    nc.all_engine_barrier()
    assert self.sems is not None
    popped = nc._tile_sem_poison_stack.pop()
    assert popped is self._sem_poison
    nc.clear_and_free_semaphores(list(self.sems.allocated().values()))


tile.TileContext._drain_and_barrier = _patched_drain_and_barrier

_WAIT_CAP = {"EventSemaphore": 2}  # walrus wait-slot capacity by opcode
_wsplit_n = 0


def _split_multi_waits(nc: bass.Bass):
    """This walrus build allows 1 sync wait per instruction (2 for
    EventSemaphore), but Tile emits instructions with up to 3.  Move excess
    waits onto NoOps inserted just before the instruction on the same engine
    queue — identical ordering semantics, codegen-legal."""
    global _wsplit_n
    for bb in nc.m.functions[0].blocks:
        out = []
        for inst in bb.instructions:
            si = inst.sync_info
            waits = list(si.on_wait) if si is not None and si.on_wait else []
            cap = _WAIT_CAP.get(inst.opcode, 1)
            if len(waits) > cap:
                for w in waits[:-cap]:
                    _wsplit_n += 1
                    out.append(
                        mybir.InstNoOp(
                            name=f"I-wsplit-{_wsplit_n}",
                            sync_info=mybir.SyncInfo(on_wait=[w], on_update=[]),
                            bass_nofuse=True,
                            engine=inst.engine,
                        )
                    )
                inst.sync_info = mybir.SyncInfo(
                    on_wait=waits[-cap:], on_update=list(si.on_update or [])
                )
            out.append(inst)
        bb.instructions = out


def build_nc() -> bass.Bass:
    nc = bass.Bass()
    enc_d = nc.declare_dram_parameter("encoder", [B_LOC, TE, H], F32, isOutput=False)
    dec_d = nc.declare_dram_parameter("decoder", [B_LOC, TD, H], F32, isOutput=False)
    mask_d = nc.declare_dram_parameter("mask", [B_LOC, TE], F32, isOutput=False)
    wa_d = nc.declare_dram_parameter("Wa", [H, H], F32, isOutput=False)
    out_d = nc.declare_dram_parameter("out", [B_LOC, TD, 2 * H], F32, isOutput=True)

    with tile.TileContext(nc) as tc:
        with (
            tc.tile_pool(name="consts", bufs=1) as consts,
            tc.tile_pool(name="enc_sb", bufs=6) as enc_pool,
            tc.tile_pool(name="out_sb", bufs=4) as out_pool,
            tc.tile_pool(name="tr_sb", bufs=8 * NH) as tr_pool,
            tc.tile_pool(name="projT_sb", bufs=4 * NH) as projT_pool,
            tc.tile_pool(name="expa_sb", bufs=8 * NE) as expa_pool,
            tc.tile_pool(name="small_sb", bufs=4 * ND) as small_pool,
            tc.tile_pool(name="ps_tr", bufs=3, space="PSUM") as ps_tr,
            tc.tile_pool(name="ps_proj", bufs=1, space="PSUM") as ps_proj,
            tc.tile_pool(name="ps_a", bufs=2, space="PSUM") as ps_a,
            tc.tile_pool(name="ps_c", bufs=2, space="PSUM") as ps_c,
        ):
            EW = H + 2  # 258: encoder chunk width incl. ones cols
            loaded = {}

            def stage_load(b, prio=False):
                # encoder mega-tile [128, 4*258]: chunk t at cols t*258,
                # data cols 0:256, ones at 256:258. One DMA + one strided
                # ones fill per batch.
                enc_big = enc_pool.tile([P, NE * EW], F32R, tag="enc")
                enc_v = enc_big.rearrange("p (t c) -> p t c", c=EW)
                nc.scalar.dma_start(
                    out=enc_v[:, :, 0:H],
                    in_=enc_d[b].rearrange("(t p) h -> p t h", p=P).bitcast(F32R),
                )
                # output mega-tile [128, 4*512]: d-tile at cols d*512; right
                # half of each = decoder (the concat).
                out_big = out_pool.tile([P, ND * 2 * H], F32R, tag="out")
                out_v = out_big.rearrange("p (d c) -> p d c", c=2 * H)
                nc.sync.dma_start(
                    out=out_v[:, :, H : 2 * H],
                    in_=dec_d[b].rearrange("(d p) h -> p d h", p=P).bitcast(F32R),
                )
                loaded[b] = (enc_big, out_big)

            stage_load(0, prio=True)
            stage_load(1)

            # ---- constants ----
            ident = consts.tile([P, P], F32, tag="ident")
            make_identity(nc, ident)
            # f32r variants (GPSIMD memset/affine_select can't emit f32r;
            # DVE copy rounds f32 -> f32r legally)
            ident_r = consts.tile([P, P], F32R, tag="ident_r")
            nc.vector.tensor_copy(out=ident_r, in_=ident)
            ones_f = consts.tile([P, 2 * NE], F32, tag="ones_f")
            nc.gpsimd.memset(ones_f, 1.0)
            ones8_r = consts.tile([P, 2 * NE], F32R, tag="ones8_r")
            nc.vector.tensor_copy(out=ones8_r, in_=ones_f)

            wa_sb = []
            for k in range(NH):
                w = consts.tile([P, H], F32R, tag=f"wa{k}")
                nc.sync.dma_start(
                    out=w, in_=wa_d[k * P : (k + 1) * P, :].bitcast(F32R)
                )
                wa_sb.append(w)

            # mask -> transposed -> ln(mask); col t*B_LOC + b holds
            # ln(mask[b, t*128 : (t+1)*128]) as a [128,1] column.
            mask_raw = consts.tile([B_LOC, TE], F32, tag="mask_raw")
            nc.scalar.dma_start(out=mask_raw, in_=mask_d[:, :])
            mask_raw = consts.tile([B_LOC, TE], F32, tag="mask_raw")
            nc.sync.dma_start(out=mask_raw, in_=mask_d[:, :])
            mask_ps = ps_tr.tile([P, NE * B_LOC], F32, tag="tr")
            for t in range(NE):
                nc.tensor.transpose(
                    out=mask_ps[:, t * B_LOC : (t + 1) * B_LOC],
                    in_=mask_raw[:, t * P : (t + 1) * P],
                    identity=ident[:B_LOC, :B_LOC],
                )
            mask_ln = consts.tile([P, NE * B_LOC], F32, tag="mask_ln")
            nc.scalar.activation(out=mask_ln, in_=mask_ps, func=ACT.Ln)
            # fold in the -64*ln2 exp shift
            nc.vector.tensor_scalar_add(out=mask_ln, in0=mask_ln, scalar1=-SHIFT)

            # ---- per-batch pipeline ----
            # Emission is software-pipelined: batch b's front end (loads,
            # transposes, proj/score matmuls, exp) is emitted before batch
            # b-1's tail (context matmul, normalize, stores), so no engine
            # queue has a late-dependency op ahead of the next batch's
            # PE-feeding work (engine queues are strict FIFO).
            state = {}

            def stage_front(b):
                enc_big, out_big = loaded.pop(b)
                enc_v = enc_big.rearrange("p (t c) -> p t c", c=EW)
                nc.vector.tensor_copy(out=enc_v[:, :, H:EW], in_=ones8_r)
                enc_t = [enc_big[:, t * EW : (t + 1) * EW] for t in range(NE)]

                # transposes: encT/decT [h, seq] as NH tiles [128, 512].
                # All in f32r views (1.5 cyc/row vs 2.0 for f32; the values
                # feed f32r matmuls and would be FP22-truncated there anyway).
                encT, decT = [], []
                for which in ("enc", "dec"):
                    for k in range(NH):
                        ps = ps_tr.tile([P, TE], F32, tag="tr")
                        for t in range(NE):
                            if which == "enc":
                                src_ap = enc_big[:, t * EW + k * P : t * EW + (k + 1) * P]
                                out_ap = ps[:, t * P : (t + 1) * P].bitcast(F32R)
                                idn = ident_r
                            else:
                                src_ap = out_big[
                                    :, t * 2 * H + H + k * P : t * 2 * H + H + (k + 1) * P
                                ]
                                out_ap = ps[:, t * P : (t + 1) * P].bitcast(F32R)
                                idn = ident_r
                            nc.tensor.transpose(out=out_ap, in_=src_ap, identity=idn)
                        sb = tr_pool.tile([P, TE], F32R, tag="tr")
                        if which == "enc":
                            nc.vector.tensor_copy(out=sb, in_=ps.bitcast(F32R))
                            encT.append(sb)
                        else:
                            nc.scalar.copy(out=sb, in_=ps.bitcast(F32R))
                            decT.append(sb)

                # projT[g, e] = Wa^T @ encT  (accumulate over h chunks)
                projT = []
                for g in range(NH):
                    pps = ps_proj.tile([P, TE], F32, tag="proj")
                    for k in range(NH):
                        nc.tensor.matmul(
                            out=pps,
                            lhsT=wa_sb[k][:, g * P : (g + 1) * P],
                            rhs=encT[k],
                            start=(k == 0),
                            stop=(k == NH - 1),
                        )
                    sb = projT_pool.tile([P, TE], F32R, tag="proj")
                    if g == 0:
                        nc.vector.tensor_copy(out=sb, in_=pps)
                    else:
                        nc.scalar.copy(out=sb, in_=pps)
                    projT.append(sb)

                # A[e, d] = projT^T @ decT;  expS = exp(A + ln(mask) - shift)
                expa = []
                for t in range(NE):
                    aps = ps_a.tile([P, TD], F32, tag="a")
                    for k in range(NH):
                        nc.tensor.matmul(
                            out=aps,
                            lhsT=projT[k][:, t * P : (t + 1) * P],
                            rhs=decT[k],
                            start=(k == 0),
                            stop=(k == NH - 1),
                        )
                    ea = expa_pool.tile([P, TD], F32R, tag="expa")
                    col = t * B_LOC + b
                    nc.scalar.activation(
                        out=ea,
                        in_=aps,
                        func=ACT.Exp,
                        bias=mask_ln[:, col : col + 1],
                    )
                    expa.append(ea)
                state[b] = (enc_t, out_big, expa)

            def stage_tail(b):
                enc_t, out_big, expa = state.pop(b)
                last = b == B_LOC - 1
                if last:
                    # the dec halves are ready as soon as the load landed;
                    # storing them now halves the end-of-kernel store burst
                    for d in range(ND):
                        nc.sync.dma_start(
                            out=out_d[b, d * P : (d + 1) * P, H : 2 * H].bitcast(F32R),
                            in_=out_big[:, d * 2 * H + H : (d + 1) * 2 * H],
                        )
                # C_un[d, 0:258] = expS^T @ [enc | 1]; normalize; emit output
                for d in range(ND):
                    cps = ps_c.tile([P, H + 2], F32, tag="c")
                    for t in range(NE):
                        nc.tensor.matmul(
                            out=cps,
                            lhsT=expa[t][:, d * P : (d + 1) * P],
                            rhs=enc_t[t],
                            start=(t == 0),
                            stop=(t == NE - 1),
                        )
                    rec = small_pool.tile([P, 1], F32, tag="rec")
                    nc.vector.reciprocal(out=rec, in_=cps[:, H : H + 1])
                    # zS = (S_s * 2^64) * 0: the first mult overflows to inf
                    # exactly where the reference's unshifted sum does, the
                    # second turns inf -> NaN and finite -> +0.
                    zs = small_pool.tile([P, 1], F32, tag="zs")
                    nc.vector.tensor_scalar(
                        out=zs,
                        in0=cps[:, H : H + 1],
                        scalar1=2.0**SHIFT_POW,
                        scalar2=0.0,
                        op0=mybir.AluOpType.mult,
                        op1=mybir.AluOpType.mult,
                    )
                    nc.vector.tensor_scalar(
                        out=out_big[:, d * 2 * H : d * 2 * H + H],
                        in0=cps[:, 0:H],
                        scalar1=rec[:, 0:1],
                        scalar2=zs[:, 0:1],
                        op0=mybir.AluOpType.mult,
                        op1=mybir.AluOpType.add,
                    )
                    if last:
                        nc.sync.dma_start(
                            out=out_d[b, d * P : (d + 1) * P, 0:H].bitcast(F32R),
                            in_=out_big[:, d * 2 * H : d * 2 * H + H],
                        )
                    else:
                        nc.sync.dma_start(
                            out=out_d[b, d * P : (d + 1) * P, :].bitcast(F32R),
                            in_=out_big[:, d * 2 * H : (d + 1) * 2 * H],
                        )

            stage_front(0)
            for b in range(1, B_LOC):
                if b + 1 < B_LOC:
                    stage_load(b + 1)
                stage_front(b)
                stage_tail(b - 1)
            stage_tail(B_LOC - 1)

    _split_multi_waits(nc)
    return nc


_NC_CACHE = None


def _get_nc():
    global _NC_CACHE
    if _NC_CACHE is None:
        _NC_CACHE = build_nc()
    return _NC_CACHE


def kernel(encoder, decoder, mask, Wa):
    encoder = np.ascontiguousarray(encoder, dtype=np.float32)
    decoder = np.ascontiguousarray(decoder, dtype=np.float32)
    mask = np.ascontiguousarray(mask, dtype=np.float32)
    Wa = np.ascontiguousarray(Wa, dtype=np.float32)

    nc = _get_nc()
    in_maps = []
    for c in range(N_CORES):
        s = slice(c * B_LOC, (c + 1) * B_LOC)
        in_maps.append(
            {
                "encoder": encoder[s],
                "decoder": decoder[s],
                "mask": mask[s],
                "Wa": Wa,
            }
        )
    res = run_bass_kernel_spmd(nc, in_maps, list(range(N_CORES)))
    return np.concatenate([res.results[c]["out"] for c in range(N_CORES)], axis=0)
